# revision 1
# baseline (speedup 1.0000x reference)
"""LEM cell (ODE2) Bass kernel for Trainium2, 8-core data-parallel.

Math (per batch row b):
  ti = x @ W_ih.T + b_ih                  # [B, 4H]
  th = y @ W_hh.T + b_hh                  # [B, 3H]
  tdt = dt @ W_dt.T + b_dt                # [B, 2]
  ms_dt_bar = sig(tdt[:,0]) * sig(ti[:, :H]   + th[:, :H])
  ms_dt     = sig(tdt[:,1]) * sig(ti[:, H:2H] + th[:, H:2H])
  z_new = (1-ms_dt) * z + ms_dt * tanh(ti[:, 3H:] + th[:, 2H:3H])
  y_new = (1-ms_dt_bar) * y + ms_dt_bar * tanh(z_new @ W_z.T + b_z + ti[:, 2H:3H])
  returns (y_new, z_new)

Strategy: shard batch across 8 cores (2048 rows each). On-chip everything is
feature-major ([feature_tile=128 partitions, batch columns free]) so no
on-chip transposes are needed: the host pre-transposes x/y/z and pre-packs
the weights into per-output-tile stationary blocks. The i+h sums and the
i_z + z_new@W_z.T sum are obtained for free by accumulating both GEMMs into
the same PSUM bank. Matmuls run as float32r (fp32 bits, full PE rate;
HW rounds internally, ~1.5e-4 rel per K=128 tile).
"""

import sys

_REPO = "/opt/trn_rl_repo"
if _REPO not in sys.path:
    sys.path.insert(0, _REPO)

from contextlib import ExitStack

import numpy as np

import concourse.bacc as bacc
import concourse.bass as bass
import concourse.tile as tile
from concourse import mybir
from concourse.bass_utils import run_bass_kernel_spmd

P = 128
F32 = mybir.dt.float32
F32R = mybir.dt.float32r
AF = mybir.ActivationFunctionType

N_CORES = 8
NINP = 1024
NHID = 1024
BATCH = 16384

LAST_RESULTS = None  # BassKernelResults of the most recent kernel() call


def build_nc(
    K,            # input feature dim (x)
    H,            # hidden dim (y/z)
    B_shard,      # batch rows per core
    panel,        # batch columns kept resident per pass
    chunk,        # matmul moving-dim size (<=512 fp32)
    wdt00, wdt10,  # W_dt scalars (baked immediates; b_dt rides in biasP)
    mm_dt=F32R,
    w_bufs=5,
    ps_bufs=8,
    xy_bufs=None,
):
    NJT = H // P          # output feature tiles (per H-sized group)
    NKT = K // P          # contraction tiles over x features
    NHT = H // P          # contraction tiles over y/z features
    npan = B_shard // panel
    nch = panel // chunk
    if xy_bufs is None:
        xy_bufs = NKT * nch + 6   # one panel's tiles + cross-panel prefetch

    def f32v(ap):
        """fp32 view of an mm-typed AP for DVE/ACT consumers."""
        return ap.bitcast(F32) if mm_dt != F32 else ap

    nc = bacc.Bacc(trn_type="TRN2", target_bir_lowering=False)

    xT = nc.declare_dram_parameter("xT", [K, B_shard], mm_dt, isOutput=False)
    yT = nc.declare_dram_parameter("yT", [H, B_shard], mm_dt, isOutput=False)
    zT = nc.declare_dram_parameter("zT", [H, B_shard], F32, isOutput=False)
    dtr = nc.declare_dram_parameter("dtr", [1, B_shard], F32, isOutput=False)
    # packed stationary blocks: [jt, kin, (kt_a*P+j | kt_b*P+j)]
    Wd2 = nc.declare_dram_parameter("Wd2", [NJT, P, K + H], mm_dt, isOutput=False)
    Wy = nc.declare_dram_parameter("Wy", [NJT, P, K + H], mm_dt, isOutput=False)
    Wd1 = nc.declare_dram_parameter("Wd1", [NJT, P, K + H], mm_dt, isOutput=False)
    Wg3 = nc.declare_dram_parameter("Wg3", [NJT, P, K + H], mm_dt, isOutput=False)
    # last two columns: row 0 holds b_dt[0], b_dt[1]
    biasP = nc.declare_dram_parameter("biasP", [P, 4 * NJT + 2], F32, isOutput=False)

    y_newT = nc.declare_dram_parameter("y_newT", [H, B_shard], F32, isOutput=True)
    z_newT = nc.declare_dram_parameter("z_newT", [H, B_shard], F32, isOutput=True)

    with tile.TileContext(nc) as tc, ExitStack() as ctx:
        cpool = ctx.enter_context(tc.tile_pool(name="cpool", bufs=1))
        xpool = ctx.enter_context(tc.tile_pool(name="xpool", bufs=xy_bufs))
        ypool = ctx.enter_context(tc.tile_pool(name="ypool", bufs=xy_bufs))
        zpool = ctx.enter_context(tc.tile_pool(name="zpool", bufs=2))
        znpool = ctx.enter_context(tc.tile_pool(name="znpool", bufs=NHT))
        wpool = ctx.enter_context(tc.tile_pool(name="wpool", bufs=w_bufs))
        apool = ctx.enter_context(tc.tile_pool(name="apool", bufs=3))
        dpool = ctx.enter_context(tc.tile_pool(name="dpool", bufs=4))
        opool = ctx.enter_context(tc.tile_pool(name="opool", bufs=2))
        bcpool = ctx.enter_context(tc.tile_pool(name="bcpool", bufs=1))
        rpool = ctx.enter_context(tc.tile_pool(name="rpool", bufs=2))
        pspool = ctx.enter_context(tc.tile_pool(name="pspool", bufs=ps_bufs, space="PSUM"))

        bias_sb = cpool.tile([P, 4 * NJT + 2], F32, name="bias_sb")
        nc.sync.dma_start(bias_sb[:], biasP[:, :])

        def bias_ap(g, jt):
            i = g * NJT + jt
            return bias_sb[:, i : i + 1]

        for p in range(npan):
            b0 = p * panel

            def col(c, n=1):
                return slice(b0 + c * chunk, b0 + (c + n) * chunk)

            # chunked input tiles; cold-start-friendly DMA order:
            # sync: dt, first weights; scalar: x(c0), y(c0), then c1
            dt_sb = rpool.tile([1, panel], F32, name="dt_sb", tag="dtr", bufs=1)
            nc.sync.dma_start(dt_sb[:], dtr[0:1, b0 : b0 + panel])

            # per-batch dt gates first: tiny ACT ops must precede the input
            # DMA flood in the ACT FIFO, else bc gates arrive ~40us late
            sg1 = rpool.tile([1, panel], F32, name="sg1", tag="sg")
            nc.scalar.activation(
                sg1[:], dt_sb[:], AF.Sigmoid,
                bias=bias_sb[0:1, 4 * NJT : 4 * NJT + 1], scale=wdt00,
            )
            sg2 = rpool.tile([1, panel], F32, name="sg2", tag="sg")
            nc.scalar.activation(
                sg2[:], dt_sb[:], AF.Sigmoid,
                bias=bias_sb[0:1, 4 * NJT + 1 : 4 * NJT + 2], scale=wdt10,
            )
            bc1 = bcpool.tile([P, panel], F32, name="bc1", tag="bc1")
            nc.gpsimd.partition_broadcast(bc1[:], sg1[0:1, :])
            bc2 = bcpool.tile([P, panel], F32, name="bc2", tag="bc2")
            nc.gpsimd.partition_broadcast(bc2[:], sg2[0:1, :])

            x_t = [[None] * nch for _ in range(NKT)]
            y_t = [[None] * nch for _ in range(NHT)]

            def load_x(kt, c):
                xt_ = xpool.tile([P, chunk], mm_dt, name="xt", tag="xt")
                nc.scalar.dma_start(xt_[:], xT[kt * P : (kt + 1) * P, col(c)])
                x_t[kt][c] = xt_

            def load_y(kt, c):
                yt_ = ypool.tile([P, chunk], mm_dt, name="yt", tag="yt")
                nc.sync.dma_start(yt_[:], yT[kt * P : (kt + 1) * P, col(c)])
                y_t[kt][c] = yt_

            def load_w(Wsrc, jt, name):
                w_sb = wpool.tile([P, K + H], mm_dt, name=name, tag="w")
                nc.sync.dma_start(w_sb[:, 0:K], Wsrc[jt][:, 0:K])
                nc.scalar.dma_start(w_sb[:, K : K + H], Wsrc[jt][:, K : K + H])
                return w_sb

            def load_w_half(Wsrc, jt, w_sb, half):
                eng = nc.sync if half == 0 else nc.scalar
                lo = 0 if half == 0 else K
                hi = K if half == 0 else K + H
                eng.dma_start(w_sb[:, lo:hi], Wsrc[jt][:, lo:hi])

            # staged cold-start: the ih halves (sync) land before y(c0),
            # the hh halves (scalar) after x(c0); two jt's worth prestaged
            n_pre = min(2, NJT)
            pre_w = []
            for jt in range(n_pre):
                wd2_sb = wpool.tile([P, K + H], mm_dt, name="wd2_sb", tag="w")
                wy_sb = wpool.tile([P, K + H], mm_dt, name="wy_sb", tag="w")
                pre_w.append((wd2_sb, wy_sb))
            for idx in range(max(n_pre, nch)):
                if idx < n_pre:
                    load_w_half(Wd2, idx, pre_w[idx][0], 0)
                    load_w_half(Wy, idx, pre_w[idx][1], 0)
                if idx < nch:
                    for kt in range(NKT):
                        load_x(kt, idx)
                if idx < n_pre:
                    load_w_half(Wd2, idx, pre_w[idx][0], 1)
                    load_w_half(Wy, idx, pre_w[idx][1], 1)
                if idx < nch:
                    for kt in range(NHT):
                        load_y(kt, idx)

            def accum_group(ps, w_sb, rhs_a, rhs_b):
                """16-matmul accumulation: sum_k Wa[k].T@a[k] + Wb[k].T@b[k]."""
                n_a = len(rhs_a)
                for kt in range(n_a):
                    nc.tensor.matmul(
                        ps[:],
                        lhsT=w_sb[:, kt * P : (kt + 1) * P],
                        rhs=rhs_a[kt][:],
                        start=(kt == 0),
                        stop=False,
                    )
                n_b = len(rhs_b)
                for kt in range(n_b):
                    nc.tensor.matmul(
                        ps[:],
                        lhsT=w_sb[:, K + kt * P : K + (kt + 1) * P],
                        rhs=rhs_b[kt][:],
                        start=False,
                        stop=(kt == n_b - 1),
                    )

            # ---- phase B: d2 + y gates -> z_new ----
            zn_t = []
            for jt in range(NJT):
                if jt < n_pre:
                    wd2_sb, wy_sb = pre_w[jt]
                else:
                    wd2_sb = load_w(Wd2, jt, "wd2_sb")
                    wy_sb = load_w(Wy, jt, "wy_sb")
                znr = znpool.tile([P, panel], mm_dt, name="znr", tag="zn")
                zn_t.append(znr)
                for c in range(nch):
                    cs = slice(c * chunk, (c + 1) * chunk)
                    z_sb = zpool.tile([P, chunk], F32, name="z_sb", tag="z")
                    nc.gpsimd.dma_start(z_sb[:], zT[jt * P : (jt + 1) * P, col(c)])

                    ps1 = pspool.tile([P, chunk], F32, name="ps1", tag="ps")
                    accum_group(ps1, wd2_sb, [x_t[k][c] for k in range(NKT)],
                                [y_t[k][c] for k in range(NHT)])
                    s2 = apool.tile([P, chunk], F32, name="s2", tag="act")
                    nc.scalar.activation(s2[:], ps1[:], AF.Sigmoid, bias=bias_ap(0, jt), scale=1.0)

                    ps2 = pspool.tile([P, chunk], F32, name="ps2", tag="ps")
                    accum_group(ps2, wy_sb, [x_t[k][c] for k in range(NKT)],
                                [y_t[k][c] for k in range(NHT)])
                    tz = apool.tile([P, chunk], F32, name="tz", tag="act")
                    nc.scalar.activation(tz[:], ps2[:], AF.Tanh, bias=bias_ap(1, jt), scale=1.0)

                    ms2 = dpool.tile([P, chunk], F32, name="ms2", tag="dve")
                    nc.vector.tensor_mul(ms2[:], s2[:], bc2[:, cs])
                    dlt = dpool.tile([P, chunk], F32, name="dlt", tag="dve")
                    nc.vector.tensor_sub(dlt[:], tz[:], z_sb[:])
                    prd = dpool.tile([P, chunk], F32, name="prd", tag="dve")
                    nc.vector.tensor_mul(prd[:], ms2[:], dlt[:])
                    znc = opool.tile([P, chunk], F32, name="znc", tag="znc")
                    nc.vector.tensor_add(znc[:], prd[:], z_sb[:])
                    nc.sync.dma_start(
                        z_newT[jt * P : (jt + 1) * P, col(c)], znc[:]
                    )
                    # rounding cast into the resident fp32r tile for GEMM3
                    nc.gpsimd.dma_start(znr[:, cs], znc[:])

            # ---- phase C: d1 gate + (i_z + z_new @ W_z.T) -> y_new ----
            for jt in range(NJT):
                wd1_sb = load_w(Wd1, jt, "wd1_sb")
                wg3_sb = load_w(Wg3, jt, "wg3_sb")
                for c in range(nch):
                    cs = slice(c * chunk, (c + 1) * chunk)
                    ps3 = pspool.tile([P, chunk], F32, name="ps3", tag="ps")
                    accum_group(ps3, wd1_sb, [x_t[k][c] for k in range(NKT)],
                                [y_t[k][c] for k in range(NHT)])
                    s1 = apool.tile([P, chunk], F32, name="s1", tag="act")
                    nc.scalar.activation(s1[:], ps3[:], AF.Sigmoid, bias=bias_ap(2, jt), scale=1.0)

                    ps4 = pspool.tile([P, chunk], F32, name="ps4", tag="ps")
                    accum_group(ps4, wg3_sb, [x_t[k][c] for k in range(NKT)],
                                [zn_t[h][:, cs] for h in range(NHT)])
                    u = apool.tile([P, chunk], F32, name="u", tag="act")
                    nc.scalar.activation(u[:], ps4[:], AF.Tanh, bias=bias_ap(3, jt), scale=1.0)

                    # yn = (y - ms1*y) + ms1*u; the first two ops only
                    # need s1, so just two DVE ops trail the final tanh
                    ms1 = dpool.tile([P, chunk], F32, name="ms1", tag="dve")
                    nc.vector.tensor_mul(ms1[:], s1[:], bc1[:, cs])
                    my = dpool.tile([P, chunk], F32, name="my", tag="dve")
                    nc.vector.tensor_mul(my[:], ms1[:], f32v(y_t[jt][c][:]))
                    wyp = dpool.tile([P, chunk], F32, name="wyp", tag="dve")
                    nc.vector.tensor_sub(wyp[:], f32v(y_t[jt][c][:]), my[:])
                    mu = dpool.tile([P, chunk], F32, name="mu", tag="dve")
                    nc.vector.tensor_mul(mu[:], ms1[:], u[:])
                    yn = opool.tile([P, chunk], F32, name="yn", tag="yn")
                    nc.vector.tensor_add(yn[:], wyp[:], mu[:])
                    nc.scalar.dma_start(
                        y_newT[jt * P : (jt + 1) * P, col(c)], yn[:]
                    )

    nc.compile()
    return nc


def _pack_pair(Wa, Wb):
    """[jt, kin, kt*P+j] stationary-block packing of two row-major [out, in]
    weight matrices (lhsT blocks: lhsT[kin, j] = W[jt*P+j, kt*P+kin])."""
    def pack(W):
        O, I = W.shape
        njt, nkt = O // P, I // P
        return (
            W.reshape(njt, P, nkt, P).transpose(0, 3, 2, 1).reshape(njt, P, I)
        )
    A = pack(Wa)
    B = pack(Wb)
    return np.ascontiguousarray(np.concatenate([A, B], axis=2), dtype=np.float32)


def pack_host_inputs(x, y, z, dt, W_ih, b_ih, W_hh, b_hh, W_z, b_z, b_dt, n_cores):
    """Shard batch across cores; pre-transpose activations; pack weights."""
    B, K = x.shape
    H = y.shape[1]
    NJT = H // P
    Bs = B // n_cores

    xT = np.ascontiguousarray(x.T)
    yT = np.ascontiguousarray(y.T)
    zT = np.ascontiguousarray(z.T)
    dtrow = np.ascontiguousarray(dt.reshape(1, B))

    Wd2 = _pack_pair(W_ih[H : 2 * H], W_hh[H : 2 * H])
    Wy = _pack_pair(W_ih[3 * H : 4 * H], W_hh[2 * H : 3 * H])
    Wd1 = _pack_pair(W_ih[0:H], W_hh[0:H])
    Wg3 = _pack_pair(W_ih[2 * H : 3 * H], W_z)

    def bias_cols(bvec):
        return bvec.reshape(NJT, P).T  # [P, NJT]

    bdt_cols = np.zeros((P, 2), np.float32)
    bdt_cols[0, 0] = b_dt[0]
    bdt_cols[0, 1] = b_dt[1]
    biasP = np.ascontiguousarray(
        np.concatenate(
            [
                bias_cols(b_ih[H : 2 * H] + b_hh[H : 2 * H]),
                bias_cols(b_ih[3 * H : 4 * H] + b_hh[2 * H : 3 * H]),
                bias_cols(b_ih[0:H] + b_hh[0:H]),
                bias_cols(b_ih[2 * H : 3 * H] + b_z),
                bdt_cols,
            ],
            axis=1,
        ),
        dtype=np.float32,
    )

    in_maps = []
    for c in range(n_cores):
        cs = slice(c * Bs, (c + 1) * Bs)
        in_maps.append(
            {
                "xT": np.ascontiguousarray(xT[:, cs]),
                "yT": np.ascontiguousarray(yT[:, cs]),
                "zT": np.ascontiguousarray(zT[:, cs]),
                "dtr": np.ascontiguousarray(dtrow[:, cs]),
                "Wd2": Wd2,
                "Wy": Wy,
                "Wd1": Wd1,
                "Wg3": Wg3,
                "biasP": biasP,
            }
        )
    return in_maps


def kernel(x, y, z, dt, W_ih, b_ih, W_hh, b_hh, W_z, b_z, W_dt, b_dt):
    x = np.asarray(x, np.float32)
    y = np.asarray(y, np.float32)
    z = np.asarray(z, np.float32)
    dt = np.asarray(dt, np.float32)
    W_ih = np.asarray(W_ih, np.float32)
    b_ih = np.asarray(b_ih, np.float32)
    W_hh = np.asarray(W_hh, np.float32)
    b_hh = np.asarray(b_hh, np.float32)
    W_z = np.asarray(W_z, np.float32)
    b_z = np.asarray(b_z, np.float32)
    W_dt = np.asarray(W_dt, np.float32)
    b_dt = np.asarray(b_dt, np.float32)

    B, K = x.shape
    H = y.shape[1]
    Bs = B // N_CORES

    in_maps = pack_host_inputs(
        x, y, z, dt, W_ih, b_ih, W_hh, b_hh, W_z, b_z, b_dt, N_CORES
    )
    nc = build_nc(
        K,
        H,
        Bs,
        panel=1024,
        chunk=512,
        wdt00=float(W_dt[0, 0]),
        wdt10=float(W_dt[1, 0]),
    )
    import os

    trace = os.environ.get("LEM_TRACE", "0") == "1"
    tmpdir = os.environ.get("LEM_TMPDIR") or None
    res = run_bass_kernel_spmd(
        nc, in_maps, list(range(N_CORES)), trace=trace, tmpdir=tmpdir
    )
    global LAST_RESULTS
    LAST_RESULTS = res
    y_newT = np.concatenate([r["y_newT"] for r in res.results], axis=1)
    z_newT = np.concatenate([r["z_newT"] for r in res.results], axis=1)
    return (
        np.ascontiguousarray(y_newT.T, dtype=np.float32),
        np.ascontiguousarray(z_newT.T, dtype=np.float32),
    )



# revision 2
# speedup vs baseline: 1.9259x; 1.9259x over previous
"""LEM cell (ODE2) Bass kernel for Trainium2, 8-core data-parallel, fp8 GEMMs.

Math (per batch row b):
  ti = x @ W_ih.T + b_ih                  # [B, 4H]
  th = y @ W_hh.T + b_hh                  # [B, 3H]
  tdt = dt @ W_dt.T + b_dt                # [B, 2]
  ms_dt_bar = sig(tdt[:,0]) * sig(ti[:, :H]   + th[:, :H])
  ms_dt     = sig(tdt[:,1]) * sig(ti[:, H:2H] + th[:, H:2H])
  z_new = (1-ms_dt) * z + ms_dt * tanh(ti[:, 3H:] + th[:, 2H:3H])
  y_new = (1-ms_dt_bar) * y + ms_dt_bar * tanh(z_new @ W_z.T + b_z + ti[:, 2H:3H])
  returns (y_new, z_new)

Strategy: shard batch across 8 cores (2048 rows each). On-chip everything is
feature-major ([feature_tile=128 partitions, batch columns free]); the host
pre-transposes activations and pre-packs weights into stationary blocks.
All GEMMs run in fp8 e4m3 with MatmulPerfMode.DoubleRow (two 128-row
contraction blocks per matmul at 0.5 PE cycles/moving-row = 2x bf16 rate).
Activations are scaled x16, weights x1024 before the e4m3 cast; the 2^-14
dequant rides for free in the PSUM-consuming activation's `scale` operand.
The i+h sums and the i_z + z_new@W_z.T sum are obtained by accumulating
both contractions into the same PSUM bank. z / y for the element-wise
paths travel separately as fp32 / bf16, outputs are fp32. Measured fp8
quantization error vs the fp32 reference: rel ~1.4e-2 (absmax metric).
"""

import sys

_REPO = "/opt/trn_rl_repo"
if _REPO not in sys.path:
    sys.path.insert(0, _REPO)

from contextlib import ExitStack

import numpy as np
import ml_dtypes

import concourse.bacc as bacc
import concourse.bass as bass
import concourse.tile as tile
from concourse import mybir
from concourse.bass_utils import run_bass_kernel_spmd

P = 128
F32 = mybir.dt.float32
F8 = mybir.dt.float8e4
BF16 = mybir.dt.bfloat16
AF = mybir.ActivationFunctionType
DR = mybir.MatmulPerfMode.DoubleRow
NP_F8 = ml_dtypes.float8_e4m3
NP_BF16 = ml_dtypes.bfloat16

N_CORES = 8
NINP = 1024
NHID = 1024
BATCH = 16384

SA = 16.0     # activation quant scale (|x|max ~5.6 -> 90 < 240)
SW = 1024.0   # weight quant scale (|w|max 1/32 -> 32 < 240)
DQ = 1.0 / (SA * SW)  # 2^-14, folded into the PSUM-reading activation

LAST_RESULTS = None  # BassKernelResults of the most recent kernel() call


def build_nc(
    K,            # input feature dim (x)
    H,            # hidden dim (y/z)
    B_shard,      # batch rows per core
    panel,        # batch columns kept resident per pass
    chunk,        # matmul moving-dim size (<=512 fp32 psum)
    wdt00, wdt10,  # W_dt scalars (baked immediates; b_dt rides in biasP)
    w_bufs=8,
    ps_bufs=8,
    xy_bufs=None,
):
    NJT = H // P          # output feature tiles (per H-sized group)
    NKP = K // (2 * P)    # x-side contraction pair-tiles (DoubleRow)
    NHP = H // (2 * P)    # y/z-side contraction pair-tiles
    NWT = (K + H) // P    # weight tile columns (packed pairs layout)
    npan = B_shard // panel
    nch = panel // chunk
    if xy_bufs is None:
        xy_bufs = NKP * nch + 4   # one panel's pair-tiles + prefetch slack

    nc = bacc.Bacc(trn_type="TRN2", target_bir_lowering=False)

    x8T = nc.declare_dram_parameter("x8T", [K, B_shard], F8, isOutput=False)
    y8T = nc.declare_dram_parameter("y8T", [H, B_shard], F8, isOutput=False)
    ybT = nc.declare_dram_parameter("ybT", [H, B_shard], BF16, isOutput=False)
    zT = nc.declare_dram_parameter("zT", [H, B_shard], F32, isOutput=False)
    dtr = nc.declare_dram_parameter("dtr", [1, B_shard], F32, isOutput=False)
    # packed stationary blocks: [jt, kin, (kt_a*P+j | kt_b*P+j)] fp8
    Wd2 = nc.declare_dram_parameter("Wd2", [NJT, P, K + H], F8, isOutput=False)
    Wy = nc.declare_dram_parameter("Wy", [NJT, P, K + H], F8, isOutput=False)
    Wd1 = nc.declare_dram_parameter("Wd1", [NJT, P, K + H], F8, isOutput=False)
    Wg3 = nc.declare_dram_parameter("Wg3", [NJT, P, K + H], F8, isOutput=False)
    # last two columns: row 0 holds b_dt[0], b_dt[1]
    biasP = nc.declare_dram_parameter("biasP", [P, 4 * NJT + 2], F32, isOutput=False)

    y_newT = nc.declare_dram_parameter("y_newT", [H, B_shard], F32, isOutput=True)
    z_newT = nc.declare_dram_parameter("z_newT", [H, B_shard], F32, isOutput=True)

    with tile.TileContext(nc) as tc, ExitStack() as ctx:
        cpool = ctx.enter_context(tc.tile_pool(name="cpool", bufs=1))
        xpool = ctx.enter_context(tc.tile_pool(name="xpool", bufs=xy_bufs))
        ypool = ctx.enter_context(tc.tile_pool(name="ypool", bufs=xy_bufs))
        ybpool = ctx.enter_context(tc.tile_pool(name="ybpool", bufs=6))
        zpool = ctx.enter_context(tc.tile_pool(name="zpool", bufs=4))
        znpool = ctx.enter_context(tc.tile_pool(name="znpool", bufs=2 * NHP))
        wpool = ctx.enter_context(tc.tile_pool(name="wpool", bufs=w_bufs))
        apool = ctx.enter_context(tc.tile_pool(name="apool", bufs=3))
        dpool = ctx.enter_context(tc.tile_pool(name="dpool", bufs=4))
        opool = ctx.enter_context(tc.tile_pool(name="opool", bufs=2))
        bcpool = ctx.enter_context(tc.tile_pool(name="bcpool", bufs=2))
        rpool = ctx.enter_context(tc.tile_pool(name="rpool", bufs=2))
        pspool = ctx.enter_context(tc.tile_pool(name="pspool", bufs=ps_bufs, space="PSUM"))

        bias_sb = cpool.tile([P, 4 * NJT + 2], F32, name="bias_sb")
        nc.sync.dma_start(bias_sb[:], biasP[:, :])

        def bias_ap(g, jt):
            i = g * NJT + jt
            return bias_sb[:, i : i + 1]

        for p in range(npan):
            b0 = p * panel

            def col(c, n=1):
                return slice(b0 + c * chunk, b0 + (c + n) * chunk)

            # chunked input tiles; cold-start-friendly DMA order:
            # sync: dt, first weights; scalar: x(c0), y(c0), then c1
            dt_sb = rpool.tile([1, panel], F32, name="dt_sb", tag="dtr", bufs=1)
            nc.sync.dma_start(dt_sb[:], dtr[0:1, b0 : b0 + panel])

            # per-batch dt gates first: tiny ACT ops must precede the input
            # DMA flood in the ACT FIFO, else bc gates arrive ~40us late
            sg1 = rpool.tile([1, panel], F32, name="sg1", tag="sg")
            nc.scalar.activation(
                sg1[:], dt_sb[:], AF.Sigmoid,
                bias=bias_sb[0:1, 4 * NJT : 4 * NJT + 1], scale=wdt00,
            )
            sg2 = rpool.tile([1, panel], F32, name="sg2", tag="sg")
            nc.scalar.activation(
                sg2[:], dt_sb[:], AF.Sigmoid,
                bias=bias_sb[0:1, 4 * NJT + 1 : 4 * NJT + 2], scale=wdt10,
            )
            bc1 = bcpool.tile([P, panel], F32, name="bc1", tag="bc1")
            nc.gpsimd.partition_broadcast(bc1[:], sg1[0:1, :])
            bc2 = bcpool.tile([P, panel], F32, name="bc2", tag="bc2")
            nc.gpsimd.partition_broadcast(bc2[:], sg2[0:1, :])

            x_t = [[None] * nch for _ in range(NKP)]
            y_t = [[None] * nch for _ in range(NHP)]

            def load_x(q, c):
                xt_ = xpool.tile([P, 2, chunk], F8, name="xt", tag="xt")
                nc.scalar.dma_start(xt_[:, 0, :], x8T[(2 * q) * P : (2 * q + 1) * P, col(c)])
                nc.scalar.dma_start(xt_[:, 1, :], x8T[(2 * q + 1) * P : (2 * q + 2) * P, col(c)])
                x_t[q][c] = xt_

            def load_y(q, c):
                yt_ = ypool.tile([P, 2, chunk], F8, name="yt", tag="yt")
                nc.sync.dma_start(yt_[:, 0, :], y8T[(2 * q) * P : (2 * q + 1) * P, col(c)])
                nc.sync.dma_start(yt_[:, 1, :], y8T[(2 * q + 1) * P : (2 * q + 2) * P, col(c)])
                y_t[q][c] = yt_

            def load_w(Wsrc, jt, name):
                w_sb = wpool.tile([P, NWT, P], F8, name=name, tag="w")
                nc.sync.dma_start(w_sb[:, 0 : K // P, :], Wsrc[jt][:, 0:K])
                nc.scalar.dma_start(w_sb[:, K // P : NWT, :], Wsrc[jt][:, K : K + H])
                return w_sb

            def load_w_half(Wsrc, jt, w_sb, half):
                eng = nc.sync if half == 0 else nc.scalar
                if half == 0:
                    eng.dma_start(w_sb[:, 0 : K // P, :], Wsrc[jt][:, 0:K])
                else:
                    eng.dma_start(w_sb[:, K // P : NWT, :], Wsrc[jt][:, K : K + H])

            # staged cold-start: the ih halves (sync) land before y(c0),
            # the hh halves (scalar) after x(c0); two jt's worth prestaged
            n_pre = min(2, NJT)
            pre_w = []
            for jt in range(n_pre):
                wd2_sb = wpool.tile([P, NWT, P], F8, name="wd2_sb", tag="w")
                wy_sb = wpool.tile([P, NWT, P], F8, name="wy_sb", tag="w")
                pre_w.append((wd2_sb, wy_sb))
            for idx in range(max(n_pre, nch)):
                if idx < n_pre:
                    load_w_half(Wd2, idx, pre_w[idx][0], 0)
                    load_w_half(Wy, idx, pre_w[idx][1], 0)
                if idx < nch:
                    for q in range(NKP):
                        load_x(q, idx)
                if idx < n_pre:
                    load_w_half(Wd2, idx, pre_w[idx][0], 1)
                    load_w_half(Wy, idx, pre_w[idx][1], 1)
                if idx < nch:
                    for q in range(NHP):
                        load_y(q, idx)

            def accum_group(ps, w_sb, rhs_a, rhs_b):
                """8-matmul fp8 DoubleRow accumulation over K=2048:
                sum_q Wa[q].T(x)a[q] + Wb[q].T(x)b[q], 256 rows per call."""
                n_a = len(rhs_a)
                for q in range(n_a):
                    nc.tensor.matmul(
                        ps[:],
                        lhsT=w_sb[:, 2 * q : 2 * q + 2, :],
                        rhs=rhs_a[q],
                        start=(q == 0),
                        stop=False,
                        perf_mode=DR,
                    )
                n_b = len(rhs_b)
                kb = K // P
                for q in range(n_b):
                    nc.tensor.matmul(
                        ps[:],
                        lhsT=w_sb[:, kb + 2 * q : kb + 2 * q + 2, :],
                        rhs=rhs_b[q],
                        start=False,
                        stop=(q == n_b - 1),
                        perf_mode=DR,
                    )

            # ---- phase B: d2 + y gates -> z_new ----
            # zn pair-tiles [P, 2, panel]: pair q holds jt=2q / 2q+1 rows
            zn_t = [
                znpool.tile([P, 2, panel], F8, name="znr", tag="zn")
                for _ in range(NHP)
            ]
            for jt in range(NJT):
                if jt < n_pre:
                    wd2_sb, wy_sb = pre_w[jt]
                else:
                    wd2_sb = load_w(Wd2, jt, "wd2_sb")
                    wy_sb = load_w(Wy, jt, "wy_sb")
                for c in range(nch):
                    cs = slice(c * chunk, (c + 1) * chunk)
                    z_sb = zpool.tile([P, chunk], F32, name="z_sb", tag="z")
                    nc.gpsimd.dma_start(z_sb[:], zT[jt * P : (jt + 1) * P, col(c)])

                    ps1 = pspool.tile([P, chunk], F32, name="ps1", tag="ps")
                    accum_group(ps1, wd2_sb, [x_t[q][c][:] for q in range(NKP)],
                                [y_t[q][c][:] for q in range(NHP)])
                    s2 = apool.tile([P, chunk], F32, name="s2", tag="act")
                    nc.scalar.activation(s2[:], ps1[:], AF.Sigmoid, bias=bias_ap(0, jt), scale=DQ)

                    ps2 = pspool.tile([P, chunk], F32, name="ps2", tag="ps")
                    accum_group(ps2, wy_sb, [x_t[q][c][:] for q in range(NKP)],
                                [y_t[q][c][:] for q in range(NHP)])
                    tz = apool.tile([P, chunk], F32, name="tz", tag="act")
                    nc.scalar.activation(tz[:], ps2[:], AF.Tanh, bias=bias_ap(1, jt), scale=DQ)

                    # z_new = z + ms2*(tanh - z); 4 DVE ops
                    ms2 = dpool.tile([P, chunk], F32, name="ms2", tag="dve")
                    nc.vector.tensor_mul(ms2[:], s2[:], bc2[:, cs])
                    dlt = dpool.tile([P, chunk], F32, name="dlt", tag="dve")
                    nc.vector.tensor_sub(dlt[:], tz[:], z_sb[:])
                    prd = dpool.tile([P, chunk], F32, name="prd", tag="dve")
                    nc.vector.tensor_mul(prd[:], ms2[:], dlt[:])
                    znc = opool.tile([P, chunk], F32, name="znc", tag="znc")
                    nc.vector.tensor_add(znc[:], prd[:], z_sb[:])
                    nc.sync.dma_start(
                        z_newT[jt * P : (jt + 1) * P, col(c)], znc[:]
                    )
                    # quantizing cast (x16 -> e4m3) into the resident
                    # DoubleRow pair-tile for GEMM3
                    nc.scalar.mul(zn_t[jt // 2][:, jt % 2, cs], znc[:], SA)

            # ---- phase C: d1 gate + (i_z + z_new @ W_z.T) -> y_new ----
            for jt in range(NJT):
                wd1_sb = load_w(Wd1, jt, "wd1_sb")
                wg3_sb = load_w(Wg3, jt, "wg3_sb")
                for c in range(nch):
                    cs = slice(c * chunk, (c + 1) * chunk)
                    yb_sb = ybpool.tile([P, chunk], BF16, name="yb_sb", tag="yb")
                    nc.gpsimd.dma_start(yb_sb[:], ybT[jt * P : (jt + 1) * P, col(c)])

                    ps3 = pspool.tile([P, chunk], F32, name="ps3", tag="ps")
                    accum_group(ps3, wd1_sb, [x_t[q][c][:] for q in range(NKP)],
                                [y_t[q][c][:] for q in range(NHP)])
                    s1 = apool.tile([P, chunk], F32, name="s1", tag="act")
                    nc.scalar.activation(s1[:], ps3[:], AF.Sigmoid, bias=bias_ap(2, jt), scale=DQ)

                    ps4 = pspool.tile([P, chunk], F32, name="ps4", tag="ps")
                    accum_group(ps4, wg3_sb, [x_t[q][c][:] for q in range(NKP)],
                                [zn_t[q][:, :, cs] for q in range(NHP)])
                    u = apool.tile([P, chunk], F32, name="u", tag="act")
                    nc.scalar.activation(u[:], ps4[:], AF.Tanh, bias=bias_ap(3, jt), scale=DQ)

                    # y_new = y + ms1*(u - y); 4 DVE ops
                    ms1 = dpool.tile([P, chunk], F32, name="ms1", tag="dve")
                    nc.vector.tensor_mul(ms1[:], s1[:], bc1[:, cs])
                    dly = dpool.tile([P, chunk], F32, name="dly", tag="dve")
                    nc.vector.tensor_sub(dly[:], u[:], yb_sb[:])
                    mdy = dpool.tile([P, chunk], F32, name="mdy", tag="dve")
                    nc.vector.tensor_mul(mdy[:], ms1[:], dly[:])
                    yn = opool.tile([P, chunk], F32, name="yn", tag="yn")
                    nc.vector.tensor_add(yn[:], mdy[:], yb_sb[:])
                    nc.scalar.dma_start(
                        y_newT[jt * P : (jt + 1) * P, col(c)], yn[:]
                    )

    nc.compile()
    return nc


def _q8(a, s):
    """Scale and round-to-nearest cast to e4m3."""
    return (np.asarray(a, np.float32) * s).astype(NP_F8)


def _pack_pair(Wa, Wb):
    """[jt, kin, kt*P+j] stationary-block packing of two row-major [out, in]
    weight matrices (lhsT blocks: lhsT[kin, j] = W[jt*P+j, kt*P+kin]).
    Consecutive kt pairs are exactly the DoubleRow [kin, 2, j] layout."""
    def pack(W):
        O, I = W.shape
        njt, nkt = O // P, I // P
        return (
            W.reshape(njt, P, nkt, P).transpose(0, 3, 2, 1).reshape(njt, P, I)
        )
    A = pack(Wa)
    B = pack(Wb)
    return np.ascontiguousarray(np.concatenate([A, B], axis=2))


def pack_host_inputs(x, y, z, dt, W_ih, b_ih, W_hh, b_hh, W_z, b_z, b_dt, n_cores):
    """Shard batch across cores; pre-transpose + fp8-quantize activations;
    pack + fp8-quantize weights."""
    B, K = x.shape
    H = y.shape[1]
    NJT = H // P
    Bs = B // n_cores

    x8T = np.ascontiguousarray(_q8(x, SA).T)
    y8T = np.ascontiguousarray(_q8(y, SA).T)
    ybT = np.ascontiguousarray(np.asarray(y, np.float32).T.astype(NP_BF16))
    zT = np.ascontiguousarray(np.asarray(z, np.float32).T)
    dtrow = np.ascontiguousarray(np.asarray(dt, np.float32).reshape(1, B))

    Wd2 = _pack_pair(_q8(W_ih[H : 2 * H], SW), _q8(W_hh[H : 2 * H], SW))
    Wy = _pack_pair(_q8(W_ih[3 * H : 4 * H], SW), _q8(W_hh[2 * H : 3 * H], SW))
    Wd1 = _pack_pair(_q8(W_ih[0:H], SW), _q8(W_hh[0:H], SW))
    Wg3 = _pack_pair(_q8(W_ih[2 * H : 3 * H], SW), _q8(W_z, SW))

    def bias_cols(bvec):
        return bvec.reshape(NJT, P).T  # [P, NJT]

    bdt_cols = np.zeros((P, 2), np.float32)
    bdt_cols[0, 0] = b_dt[0]
    bdt_cols[0, 1] = b_dt[1]
    biasP = np.ascontiguousarray(
        np.concatenate(
            [
                bias_cols(b_ih[H : 2 * H] + b_hh[H : 2 * H]),
                bias_cols(b_ih[3 * H : 4 * H] + b_hh[2 * H : 3 * H]),
                bias_cols(b_ih[0:H] + b_hh[0:H]),
                bias_cols(b_ih[2 * H : 3 * H] + b_z),
                bdt_cols,
            ],
            axis=1,
        ),
        dtype=np.float32,
    )

    in_maps = []
    for c in range(n_cores):
        cs = slice(c * Bs, (c + 1) * Bs)
        in_maps.append(
            {
                "x8T": np.ascontiguousarray(x8T[:, cs]),
                "y8T": np.ascontiguousarray(y8T[:, cs]),
                "ybT": np.ascontiguousarray(ybT[:, cs]),
                "zT": np.ascontiguousarray(zT[:, cs]),
                "dtr": np.ascontiguousarray(dtrow[:, cs]),
                "Wd2": Wd2,
                "Wy": Wy,
                "Wd1": Wd1,
                "Wg3": Wg3,
                "biasP": biasP,
            }
        )
    return in_maps


def kernel(x, y, z, dt, W_ih, b_ih, W_hh, b_hh, W_z, b_z, W_dt, b_dt):
    x = np.asarray(x, np.float32)
    y = np.asarray(y, np.float32)
    z = np.asarray(z, np.float32)
    dt = np.asarray(dt, np.float32)
    W_ih = np.asarray(W_ih, np.float32)
    b_ih = np.asarray(b_ih, np.float32)
    W_hh = np.asarray(W_hh, np.float32)
    b_hh = np.asarray(b_hh, np.float32)
    W_z = np.asarray(W_z, np.float32)
    b_z = np.asarray(b_z, np.float32)
    W_dt = np.asarray(W_dt, np.float32)
    b_dt = np.asarray(b_dt, np.float32)

    B, K = x.shape
    H = y.shape[1]
    Bs = B // N_CORES

    in_maps = pack_host_inputs(
        x, y, z, dt, W_ih, b_ih, W_hh, b_hh, W_z, b_z, b_dt, N_CORES
    )
    nc = build_nc(
        K,
        H,
        Bs,
        panel=2048,
        chunk=512,
        wdt00=float(W_dt[0, 0]),
        wdt10=float(W_dt[1, 0]),
    )
    import os

    trace = os.environ.get("LEM_TRACE", "0") == "1"
    tmpdir = os.environ.get("LEM_TMPDIR") or None
    res = run_bass_kernel_spmd(
        nc, in_maps, list(range(N_CORES)), trace=trace, tmpdir=tmpdir
    )
    global LAST_RESULTS
    LAST_RESULTS = res
    y_newT = np.concatenate([r["y_newT"] for r in res.results], axis=1)
    z_newT = np.concatenate([r["z_newT"] for r in res.results], axis=1)
    return (
        np.ascontiguousarray(y_newT.T, dtype=np.float32),
        np.ascontiguousarray(z_newT.T, dtype=np.float32),
    )


# revision 7
# speedup vs baseline: 1.9726x; 1.0243x over previous
"""LEM cell (ODE2) Bass kernel for Trainium2, 8-core data-parallel, fp8 GEMMs.

Math (per batch row b):
  ti = x @ W_ih.T + b_ih                  # [B, 4H]
  th = y @ W_hh.T + b_hh                  # [B, 3H]
  tdt = dt @ W_dt.T + b_dt                # [B, 2]
  ms_dt_bar = sig(tdt[:,0]) * sig(ti[:, :H]   + th[:, :H])
  ms_dt     = sig(tdt[:,1]) * sig(ti[:, H:2H] + th[:, H:2H])
  z_new = (1-ms_dt) * z + ms_dt * tanh(ti[:, 3H:] + th[:, 2H:3H])
  y_new = (1-ms_dt_bar) * y + ms_dt_bar * tanh(z_new @ W_z.T + b_z + ti[:, 2H:3H])
  returns (y_new, z_new)

Strategy: shard batch across 8 cores (2048 rows each). On-chip everything is
feature-major ([feature_tile=128 partitions, batch columns free]); the host
pre-transposes activations and pre-packs weights into stationary blocks.
All GEMMs run in fp8 e4m3 with MatmulPerfMode.DoubleRow (two 128-row
contraction blocks per matmul, 2x bf16 PE rate). Activations are scaled
x16, weights x1024 before the e4m3 cast; the 2^-14 dequant rides in the
PSUM-consuming activation's `scale` operand. The i+h sums and the
i_z + z_new@W_z.T sum accumulate into the same PSUM bank. z / y for the
element-wise paths travel separately as fp32 / bf16, outputs fp32.

DMA descriptor generation costs ~620ns of sequencer time per dma_start,
so transfers are merged aggressively: x/y are host-packed pair-major
([pair, 128, 2, Bs]) so one DMA fills a DoubleRow rhs tile, weights load
as single full stationary tiles, z / y-elementwise load per-jt rows, and
outputs accumulate in half-panel SBUF tiles before one store each.
Measured fp8 error vs the fp32 reference: rel ~1.4e-2 (absmax metric).
"""

import sys

_REPO = "/opt/trn_rl_repo"
if _REPO not in sys.path:
    sys.path.insert(0, _REPO)

from contextlib import ExitStack

import numpy as np
import ml_dtypes

import concourse.bacc as bacc
import concourse.bass as bass
import concourse.tile as tile
from concourse import mybir
from concourse.bass_utils import run_bass_kernel_spmd

P = 128
F32 = mybir.dt.float32
F8 = mybir.dt.float8e4
BF16 = mybir.dt.bfloat16
AF = mybir.ActivationFunctionType
DR = mybir.MatmulPerfMode.DoubleRow
NP_F8 = ml_dtypes.float8_e4m3
NP_BF16 = ml_dtypes.bfloat16

N_CORES = 8
NINP = 1024
NHID = 1024
BATCH = 16384

SA = 16.0     # activation quant scale (|x|max ~5.6 -> 90 < 240)
SW = 1024.0   # weight quant scale (|w|max 1/32 -> 32 < 240)
DQ = 1.0 / (SA * SW)  # 2^-14, folded into the PSUM-reading activation

LAST_RESULTS = None  # BassKernelResults of the most recent kernel() call


def build_nc(
    K,            # input feature dim (x)
    H,            # hidden dim (y/z)
    B_shard,      # batch rows per core
    panel,        # batch columns kept resident per pass (== B_shard here)
    chunk,        # matmul moving-dim size (512 = one fp32 PSUM bank)
    wdt00, wdt10,  # W_dt scalars (baked immediates; b_dt rides in biasP)
    w_bufs=24,
    ps_bufs=8,
):
    NJT = H // P          # output feature tiles (per H-sized group)
    NKP = K // (2 * P)    # x-side contraction pair-tiles (DoubleRow)
    NHP = H // (2 * P)    # y/z-side contraction pair-tiles
    NWT = (K + H) // P    # weight tile columns (packed pairs layout)
    npan = B_shard // panel
    nch = panel // chunk
    half = panel // 2     # output tiles cover half panels

    nc = bacc.Bacc(trn_type="TRN2", target_bir_lowering=False)

    # pair-major activations: [pair, 128, 2, B] so one DMA fills a rhs tile
    x8P = nc.declare_dram_parameter("x8P", [NKP, P, 2, B_shard], F8, isOutput=False)
    y8P = nc.declare_dram_parameter("y8P", [NHP, P, 2, B_shard], F8, isOutput=False)
    ybT = nc.declare_dram_parameter("ybT", [H, B_shard], BF16, isOutput=False)
    zT = nc.declare_dram_parameter("zT", [H, B_shard], F32, isOutput=False)
    dtr = nc.declare_dram_parameter("dtr", [1, B_shard], F32, isOutput=False)
    # packed stationary blocks: [jt, kin, (kt_a*P+j | kt_b*P+j)] fp8
    Wd2 = nc.declare_dram_parameter("Wd2", [NJT, P, K + H], F8, isOutput=False)
    Wy = nc.declare_dram_parameter("Wy", [NJT, P, K + H], F8, isOutput=False)
    Wd1 = nc.declare_dram_parameter("Wd1", [NJT, P, K + H], F8, isOutput=False)
    Wg3 = nc.declare_dram_parameter("Wg3", [NJT, P, K + H], F8, isOutput=False)
    # last two columns: row 0 holds b_dt[0], b_dt[1]
    biasP = nc.declare_dram_parameter("biasP", [P, 4 * NJT + 2], F32, isOutput=False)

    y_newT = nc.declare_dram_parameter("y_newT", [H, B_shard], F32, isOutput=True)
    z_newT = nc.declare_dram_parameter("z_newT", [H, B_shard], F32, isOutput=True)

    assert npan == 1, "single-panel schedule (whole shard resident)"

    with tile.TileContext(nc) as tc, ExitStack() as ctx:
        cpool = ctx.enter_context(tc.tile_pool(name="cpool", bufs=1))
        xpool = ctx.enter_context(tc.tile_pool(name="xpool", bufs=NKP * nch))
        ypool = ctx.enter_context(tc.tile_pool(name="ypool", bufs=NHP * nch))
        ybpool = ctx.enter_context(tc.tile_pool(name="ybpool", bufs=2))
        zpool = ctx.enter_context(tc.tile_pool(name="zpool", bufs=2))
        znpool = ctx.enter_context(tc.tile_pool(name="znpool", bufs=NHP))
        wpool = ctx.enter_context(tc.tile_pool(name="wpool", bufs=w_bufs))
        apool = ctx.enter_context(tc.tile_pool(name="apool", bufs=3))
        dpool = ctx.enter_context(tc.tile_pool(name="dpool", bufs=3))
        opool = ctx.enter_context(tc.tile_pool(name="opool", bufs=2))
        bcpool = ctx.enter_context(tc.tile_pool(name="bcpool", bufs=1))
        rpool = ctx.enter_context(tc.tile_pool(name="rpool", bufs=2))
        pspool = ctx.enter_context(tc.tile_pool(name="pspool", bufs=ps_bufs, space="PSUM"))

        bias_sb = cpool.tile([P, 4 * NJT + 2], F32, name="bias_sb")
        nc.sync.dma_start(bias_sb[:], biasP[:, :])

        def bias_ap(g, jt):
            i = g * NJT + jt
            return bias_sb[:, i : i + 1]

        b0 = 0

        def col(c, n=1):
            return slice(b0 + c * chunk, b0 + (c + n) * chunk)

        dt_sb = rpool.tile([1, panel], F32, name="dt_sb", tag="dtr", bufs=1)
        nc.sync.dma_start(dt_sb[:], dtr[0:1, b0 : b0 + panel])

        # per-batch dt gates first: tiny ACT ops must precede the input
        # DMA flood in the ACT FIFO, else bc gates arrive ~40us late
        sg1 = rpool.tile([1, panel], F32, name="sg1", tag="sg")
        nc.scalar.activation(
            sg1[:], dt_sb[:], AF.Sigmoid,
            bias=bias_sb[0:1, 4 * NJT : 4 * NJT + 1], scale=wdt00,
        )
        sg2 = rpool.tile([1, panel], F32, name="sg2", tag="sg")
        nc.scalar.activation(
            sg2[:], dt_sb[:], AF.Sigmoid,
            bias=bias_sb[0:1, 4 * NJT + 1 : 4 * NJT + 2], scale=wdt10,
        )

        x_t = [[None] * nch for _ in range(NKP)]
        y_t = [[None] * nch for _ in range(NHP)]
        z_t = [None] * NJT
        yb_t = [None] * NJT

        def load_x(q, c, eng):
            xt_ = xpool.tile([P, 2, chunk], F8, name="xt", tag="xt")
            eng.dma_start(xt_[:], x8P[q][:, :, col(c)])
            x_t[q][c] = xt_

        def load_y(q, c, eng):
            yt_ = ypool.tile([P, 2, chunk], F8, name="yt", tag="yt")
            eng.dma_start(yt_[:], y8P[q][:, :, col(c)])
            y_t[q][c] = yt_

        def load_z(jt, eng):
            z_sb = zpool.tile([P, panel], F32, name="z_sb", tag="z")
            eng.dma_start(z_sb[:], zT[jt * P : (jt + 1) * P, b0 : b0 + panel])
            z_t[jt] = z_sb

        def load_yb(jt, eng):
            yb_sb = ybpool.tile([P, panel], BF16, name="yb_sb", tag="yb")
            eng.dma_start(yb_sb[:], ybT[jt * P : (jt + 1) * P, b0 : b0 + panel])
            yb_t[jt] = yb_sb

        def load_w(Wsrc, jt, name, eng):
            w_sb = wpool.tile([P, NWT, P], F8, name=name, tag="w")
            eng.dma_start(w_sb[:], Wsrc[jt][:, :])
            return w_sb

        # ---- cold-start staging ----
        # gpsimd: x(c0) + z0, then bc broadcasts (needed by first DVE ~15us)
        for q in range(NKP):
            load_x(q, 0, nc.gpsimd)
        load_z(0, nc.gpsimd)
        bc1 = bcpool.tile([P, panel], F32, name="bc1", tag="bc1")
        nc.gpsimd.partition_broadcast(bc1[:], sg1[0:1, :])
        bc2 = bcpool.tile([P, panel], F32, name="bc2", tag="bc2")
        nc.gpsimd.partition_broadcast(bc2[:], sg2[0:1, :])
        load_z(1, nc.gpsimd)

        # sync: first stationary tile, then y(c0); scalar: the y-gate tile
        wB = [[None, None] for _ in range(NJT)]   # per jt: [Wd2, Wy]
        wC = [[None, None] for _ in range(NJT)]   # per jt: [Wd1, Wg3]
        wB[0][0] = load_w(Wd2, 0, "wd2_sb", nc.sync)
        wB[0][1] = load_w(Wy, 0, "wy_sb", nc.scalar)
        for q in range(NHP):
            load_y(q, 0, nc.sync)

        # remaining inputs + all phase-B weights, priority-interleaved
        for c in range(1, nch):
            for q in range(NKP):
                load_x(q, c, nc.scalar)
            for q in range(NHP):
                load_y(q, c, nc.sync)
            jt = c  # stage weights jt=1..3 between input chunks
            wB[jt][0] = load_w(Wd2, jt, "wd2_sb", nc.sync)
            wB[jt][1] = load_w(Wy, jt, "wy_sb", nc.scalar)
        for jt in range(nch, NJT):
            wB[jt][0] = load_w(Wd2, jt, "wd2_sb", nc.sync)
            wB[jt][1] = load_w(Wy, jt, "wy_sb", nc.scalar)

        def accum_group(ps, w_sb, rhs_a, rhs_b):
            """8-matmul fp8 DoubleRow accumulation over K=2048:
            sum_q Wa[q].T(x)a[q] + Wb[q].T(x)b[q], 256 rows per call."""
            n_a = len(rhs_a)
            for q in range(n_a):
                nc.tensor.matmul(
                    ps[:],
                    lhsT=w_sb[:, 2 * q : 2 * q + 2, :],
                    rhs=rhs_a[q],
                    start=(q == 0),
                    stop=False,
                    perf_mode=DR,
                )
            n_b = len(rhs_b)
            kb = K // P
            for q in range(n_b):
                nc.tensor.matmul(
                    ps[:],
                    lhsT=w_sb[:, kb + 2 * q : kb + 2 * q + 2, :],
                    rhs=rhs_b[q],
                    start=False,
                    stop=(q == n_b - 1),
                    perf_mode=DR,
                )

        # ---- phase B: d2 + y gates -> z_new ----
        # zn pair-tiles [P, 2, panel]: pair q holds jt=2q / 2q+1 rows
        zn_t = [
            znpool.tile([P, 2, panel], F8, name="znr", tag="zn")
            for _ in range(NHP)
        ]
        for jt in range(NJT):
            wd2_sb, wy_sb = wB[jt]
            if jt + 2 < NJT:
                load_z(jt + 2, nc.gpsimd)
            elif jt + 2 - NJT < NJT:
                load_yb(jt + 2 - NJT, nc.gpsimd)  # yb0/yb1 prefetch late in B
            # stage phase-C stationary tiles through B's back half
            if 2 <= jt < 2 + NJT // 2:
                j2 = 2 * (jt - 2)
                wC[j2][0] = load_w(Wd1, j2, "wd1_sb", nc.sync)
                wC[j2][1] = load_w(Wg3, j2, "wg3_sb", nc.scalar)
                wC[j2 + 1][0] = load_w(Wd1, j2 + 1, "wd1_sb", nc.sync)
                wC[j2 + 1][1] = load_w(Wg3, j2 + 1, "wg3_sb", nc.scalar)
            zo = opool.tile([P, panel], F32, name="zo", tag="zo")
            for c in range(nch):
                cs = slice(c * chunk, (c + 1) * chunk)
                ps1 = pspool.tile([P, chunk], F32, name="ps1", tag="ps")
                accum_group(ps1, wd2_sb, [x_t[q][c][:] for q in range(NKP)],
                            [y_t[q][c][:] for q in range(NHP)])
                s2 = apool.tile([P, chunk], F32, name="s2", tag="act")
                nc.scalar.activation(s2[:], ps1[:], AF.Sigmoid, bias=bias_ap(0, jt), scale=DQ)

                ps2 = pspool.tile([P, chunk], F32, name="ps2", tag="ps")
                accum_group(ps2, wy_sb, [x_t[q][c][:] for q in range(NKP)],
                            [y_t[q][c][:] for q in range(NHP)])
                tz = apool.tile([P, chunk], F32, name="tz", tag="act")
                nc.scalar.activation(tz[:], ps2[:], AF.Tanh, bias=bias_ap(1, jt), scale=DQ)

                # z_new = z + ms2*(tanh - z); 4 DVE ops
                ms2 = dpool.tile([P, chunk], F32, name="ms2", tag="dve")
                nc.vector.tensor_mul(ms2[:], s2[:], bc2[:, cs])
                dlt = dpool.tile([P, chunk], F32, name="dlt", tag="dve")
                nc.vector.tensor_sub(dlt[:], tz[:], z_t[jt][:, cs])
                prd = dpool.tile([P, chunk], F32, name="prd", tag="dve")
                nc.vector.tensor_mul(prd[:], ms2[:], dlt[:])
                nc.vector.tensor_add(zo[:, cs], prd[:], z_t[jt][:, cs])
                # quantizing cast (x16 -> e4m3) into the resident
                # DoubleRow pair-tile for GEMM3
                nc.scalar.mul(zn_t[jt // 2][:, jt % 2, cs], zo[:, cs], SA)
                if c % 2 == 1:
                    nc.sync.dma_start(
                        z_newT[jt * P : (jt + 1) * P, col(c - 1, 2)],
                        zo[:, (c - 1) * chunk : (c + 1) * chunk],
                    )

        # ---- phase C: d1 gate + (i_z + z_new @ W_z.T) -> y_new ----
        for jt in range(NJT):
            if wC[jt][0] is None:
                wC[jt][0] = load_w(Wd1, jt, "wd1_sb", nc.sync)
                wC[jt][1] = load_w(Wg3, jt, "wg3_sb", nc.scalar)
            wd1_sb, wg3_sb = wC[jt]
            if jt + 2 < NJT:
                load_yb(jt + 2, nc.gpsimd)
            yo = opool.tile([P, panel], F32, name="yo", tag="yo")
            for c in range(nch):
                cs = slice(c * chunk, (c + 1) * chunk)
                ps3 = pspool.tile([P, chunk], F32, name="ps3", tag="ps")
                accum_group(ps3, wd1_sb, [x_t[q][c][:] for q in range(NKP)],
                            [y_t[q][c][:] for q in range(NHP)])
                s1 = apool.tile([P, chunk], F32, name="s1", tag="act")
                nc.scalar.activation(s1[:], ps3[:], AF.Sigmoid, bias=bias_ap(2, jt), scale=DQ)

                ps4 = pspool.tile([P, chunk], F32, name="ps4", tag="ps")
                accum_group(ps4, wg3_sb, [x_t[q][c][:] for q in range(NKP)],
                            [zn_t[q][:, :, cs] for q in range(NHP)])
                u = apool.tile([P, chunk], F32, name="u", tag="act")
                nc.scalar.activation(u[:], ps4[:], AF.Tanh, bias=bias_ap(3, jt), scale=DQ)

                # y_new = y + ms1*(u - y); 4 DVE ops
                ms1 = dpool.tile([P, chunk], F32, name="ms1", tag="dve")
                nc.vector.tensor_mul(ms1[:], s1[:], bc1[:, cs])
                dly = dpool.tile([P, chunk], F32, name="dly", tag="dve")
                nc.vector.tensor_sub(dly[:], u[:], yb_t[jt][:, cs])
                mdy = dpool.tile([P, chunk], F32, name="mdy", tag="dve")
                nc.vector.tensor_mul(mdy[:], ms1[:], dly[:])
                nc.vector.tensor_add(yo[:, cs], mdy[:], yb_t[jt][:, cs])
                if c % 2 == 1:
                    nc.scalar.dma_start(
                        y_newT[jt * P : (jt + 1) * P, col(c - 1, 2)],
                        yo[:, (c - 1) * chunk : (c + 1) * chunk],
                    )

    nc.compile()
    return nc


def _q8(a, s):
    """Scale and round-to-nearest cast to e4m3."""
    return (np.asarray(a, np.float32) * s).astype(NP_F8)


def _pack_pair(Wa, Wb):
    """[jt, kin, kt*P+j] stationary-block packing of two row-major [out, in]
    weight matrices (lhsT blocks: lhsT[kin, j] = W[jt*P+j, kt*P+kin]).
    Consecutive kt pairs are exactly the DoubleRow [kin, 2, j] layout."""
    def pack(W):
        O, I = W.shape
        njt, nkt = O // P, I // P
        return (
            W.reshape(njt, P, nkt, P).transpose(0, 3, 2, 1).reshape(njt, P, I)
        )
    A = pack(Wa)
    B = pack(Wb)
    return np.ascontiguousarray(np.concatenate([A, B], axis=2))


def _pair_major(a8T):
    """[K, B] fp8 -> [K/256, 128, 2, B] DoubleRow pair-major layout."""
    K, B = a8T.shape
    return np.ascontiguousarray(
        a8T.reshape(K // 256, 2, P, B).transpose(0, 2, 1, 3)
    )


def pack_host_inputs(x, y, z, dt, W_ih, b_ih, W_hh, b_hh, W_z, b_z, b_dt, n_cores):
    """Shard batch across cores; pre-transpose + fp8-quantize activations;
    pack + fp8-quantize weights."""
    B, K = x.shape
    H = y.shape[1]
    NJT = H // P
    Bs = B // n_cores

    x8T = np.ascontiguousarray(_q8(x, SA).T)
    y8T = np.ascontiguousarray(_q8(y, SA).T)
    ybT = np.ascontiguousarray(np.asarray(y, np.float32).T.astype(NP_BF16))
    zT = np.ascontiguousarray(np.asarray(z, np.float32).T)
    dtrow = np.ascontiguousarray(np.asarray(dt, np.float32).reshape(1, B))

    Wd2 = _pack_pair(_q8(W_ih[H : 2 * H], SW), _q8(W_hh[H : 2 * H], SW))
    Wy = _pack_pair(_q8(W_ih[3 * H : 4 * H], SW), _q8(W_hh[2 * H : 3 * H], SW))
    Wd1 = _pack_pair(_q8(W_ih[0:H], SW), _q8(W_hh[0:H], SW))
    Wg3 = _pack_pair(_q8(W_ih[2 * H : 3 * H], SW), _q8(W_z, SW))

    def bias_cols(bvec):
        return bvec.reshape(NJT, P).T  # [P, NJT]

    bdt_cols = np.zeros((P, 2), np.float32)
    bdt_cols[0, 0] = b_dt[0]
    bdt_cols[0, 1] = b_dt[1]
    biasP = np.ascontiguousarray(
        np.concatenate(
            [
                bias_cols(b_ih[H : 2 * H] + b_hh[H : 2 * H]),
                bias_cols(b_ih[3 * H : 4 * H] + b_hh[2 * H : 3 * H]),
                bias_cols(b_ih[0:H] + b_hh[0:H]),
                bias_cols(b_ih[2 * H : 3 * H] + b_z),
                bdt_cols,
            ],
            axis=1,
        ),
        dtype=np.float32,
    )

    in_maps = []
    for c in range(n_cores):
        cs = slice(c * Bs, (c + 1) * Bs)
        in_maps.append(
            {
                "x8P": _pair_major(x8T[:, cs]),
                "y8P": _pair_major(y8T[:, cs]),
                "ybT": np.ascontiguousarray(ybT[:, cs]),
                "zT": np.ascontiguousarray(zT[:, cs]),
                "dtr": np.ascontiguousarray(dtrow[:, cs]),
                "Wd2": Wd2,
                "Wy": Wy,
                "Wd1": Wd1,
                "Wg3": Wg3,
                "biasP": biasP,
            }
        )
    return in_maps


def kernel(x, y, z, dt, W_ih, b_ih, W_hh, b_hh, W_z, b_z, W_dt, b_dt):
    x = np.asarray(x, np.float32)
    y = np.asarray(y, np.float32)
    z = np.asarray(z, np.float32)
    dt = np.asarray(dt, np.float32)
    W_ih = np.asarray(W_ih, np.float32)
    b_ih = np.asarray(b_ih, np.float32)
    W_hh = np.asarray(W_hh, np.float32)
    b_hh = np.asarray(b_hh, np.float32)
    W_z = np.asarray(W_z, np.float32)
    b_z = np.asarray(b_z, np.float32)
    W_dt = np.asarray(W_dt, np.float32)
    b_dt = np.asarray(b_dt, np.float32)

    B, K = x.shape
    H = y.shape[1]
    Bs = B // N_CORES

    in_maps = pack_host_inputs(
        x, y, z, dt, W_ih, b_ih, W_hh, b_hh, W_z, b_z, b_dt, N_CORES
    )
    nc = build_nc(
        K,
        H,
        Bs,
        panel=Bs,
        chunk=512,
        wdt00=float(W_dt[0, 0]),
        wdt10=float(W_dt[1, 0]),
    )
    import os

    trace = os.environ.get("LEM_TRACE", "0") == "1"
    tmpdir = os.environ.get("LEM_TMPDIR") or None
    res = run_bass_kernel_spmd(
        nc, in_maps, list(range(N_CORES)), trace=trace, tmpdir=tmpdir
    )
    global LAST_RESULTS
    LAST_RESULTS = res
    y_newT = np.concatenate([r["y_newT"] for r in res.results], axis=1)
    z_newT = np.concatenate([r["z_newT"] for r in res.results], axis=1)
    return (
        np.ascontiguousarray(y_newT.T, dtype=np.float32),
        np.ascontiguousarray(z_newT.T, dtype=np.float32),
    )
